# revision 1
# baseline (speedup 1.0000x reference)
"""Multi-head attention kernel for Trainium2, 8 NeuronCores.

Problem: B=2, S=2048, D=1024, H=16, Dk=64, fp32.
  qkv = x @ W_qkv + b_qkv ; per-head scaled-dot-product attention with
  key mask; out = attn_out @ W_out + b_out.

Sharding: DP over batch (2) x TP over head groups (4 groups of 4 heads).
Core c -> (b = c // 4, g = c % 4). Each core computes the partial output
  y_partial[b] = attn_out[:, heads(g)] @ W_out[rows(g)] (+ b_out on g==0)
and the host sums the 4 partials per batch (row-parallel unshard).

Per-core device algorithm (matmuls in float32r ~ TF32-precision at full
PE rate, fp32 PSUM accumulation; measured end-to-end rel err ~1.2e-4):
  1. x^T via PE transpose (fp32, 128x128 tiles), rounded to f32r on the
     psum->SBUF copy.
  2. Q^T, K^T = W.T @ x^T (+bias via ACT Identity), stored per head
     zero-padded to 128 contraction partitions so score matmuls stream
     at full rate (K=64 operands stream at half rate). V = x @ W_v
     (+bias), stored per key-tile as [128, head, 65] with a ones column
     appended so the attn@V matmul also produces the softmax
     denominator.
  3. Attention with heads issued in interleaved pairs (per-head PSUM
     score buffers alternate so exp on ACT overlaps the other head's
     matmuls): scoresT[key, q] = K^T.T @ Q^T (keys on partitions),
     exp((s + maskbias_key)/8) on ACT (bias is per-partition = per-key;
     no max subtraction needed: |scores/8| <= ~2), then
     outT[65, q] += V_aug.T @ expT accumulated over key tiles in PSUM.
     Normalize: outT[0:64] * (1/outT[64]) broadcast via gpsimd.
  4. y_partial = out^T.T @ W_out, DMA out. Host sums the 4 partials per
     batch and adds b_out.
"""

import numpy as np
from contextlib import ExitStack

import concourse.tile as tile
from concourse import bacc, mybir
from concourse.bass_utils import run_bass_kernel_spmd

F32 = mybir.dt.float32
F32R = mybir.dt.float32r
AF = mybir.ActivationFunctionType

S = 2048
D = 1024
H_LOC = 4           # heads per core
DK = 64
DH = H_LOC * DK     # 256: d' per core
KT = D // 128       # 8 k-tiles for the D contraction
ST = S // 128       # 16 s-tiles
SC = 4              # s super-chunks of 512
INV_SCALE = 1.0 / 8.0
ONES_F32_BITS = 0x3F800000

TRACE = False
TRACE_ALL_CORES = False
LAST_EXEC_NS = None
LAST_RESULTS = None
LAST_IN_MAPS = None

_CACHED_NC = None


def _build(phases=3, repeat=1, any_copy=True):
    nc = bacc.Bacc("TRN2", target_bir_lowering=False, debug=False,
                   enable_asserts=True, num_devices=8)

    x = nc.dram_tensor("x", [S, D], F32, kind="ExternalInput").ap()
    w_q = nc.dram_tensor("w_q", [D, DH], F32, kind="ExternalInput").ap()
    w_k = nc.dram_tensor("w_k", [D, DH], F32, kind="ExternalInput").ap()
    w_v = nc.dram_tensor("w_v", [D, DH], F32, kind="ExternalInput").ap()
    b_q = nc.dram_tensor("b_q", [DH], F32, kind="ExternalInput").ap()
    b_k = nc.dram_tensor("b_k", [DH], F32, kind="ExternalInput").ap()
    b_v = nc.dram_tensor("b_v", [DH], F32, kind="ExternalInput").ap()
    w_out = nc.dram_tensor("w_out", [DH, D], F32, kind="ExternalInput").ap()
    b_out = nc.dram_tensor("b_out", [D], F32, kind="ExternalInput").ap()
    mask_bias = nc.dram_tensor("mask_bias", [S], F32, kind="ExternalInput").ap()
    # host-prepared constants (avoid slow gpsimd Q7 setup ops on device)
    ident_in = nc.dram_tensor("ident", [128, 128], F32, kind="ExternalInput").ap()
    bv_bc_in = nc.dram_tensor("bv_bc", [128, DH], F32, kind="ExternalInput").ap()
    bout_bc_in = nc.dram_tensor("bout_bc", [128, D], F32,
                                kind="ExternalInput").ap()

    y = nc.dram_tensor("y", [S, D], F32, kind="ExternalOutput").ap()

    with tile.TileContext(nc) as tc, ExitStack() as ctx:
        if repeat > 1:
            ctx.enter_context(tc.For_i(0, repeat, 1))
        # ---------- persistent SBUF ----------
        persist = ctx.enter_context(tc.tile_pool(name="persist", bufs=1))

        # Q^T / K^T stored per head, zero-padded to 128 contraction rows:
        # head 2m+a keeps its natural partitions (a=0 -> rows 0:64 real,
        # 64:128 zero; a=1 -> rows 64:128 real, 0:64 zero) so score
        # matmuls contract over the full 128 partitions at full rate.
        qt = persist.tile([128, H_LOC, S], F32R, tag="qt")
        kt_sb = persist.tile([128, H_LOC, S], F32R, tag="kt")
        v_aug = persist.tile([128, ST, H_LOC, DK + 1], F32R, tag="vaug")
        out_ht = persist.tile([128, 2, S], F32R, tag="outht")  # attn out^T
        w_out_sb = persist.tile([128, 2, D], F32R, tag="wout")
        bq_sb = persist.tile([128, 2], F32, tag="bq")
        bk_sb = persist.tile([128, 2], F32, tag="bk")
        bv_bc = persist.tile([128, DH], F32, tag="bvbc")
        mask_sb = persist.tile([128, ST], F32, tag="mask")
        ident = persist.tile([128, 128], F32, tag="ident")

        # Padding of Q^T/K^T stripes is done by the ACT write itself:
        # scale/bias APs are 1/b on the head's real 64 partitions and 0/0
        # on the pad partitions, so one [128,512] activation writes real
        # rows and zeros together (no separate memset of the pad halves).
        sc_pad = persist.tile([128, 2], F32, tag="scpad")
        nc.vector.memset(sc_pad[:], 0.0)
        nc.vector.memset(sc_pad[0:64, 0:1], 1.0)
        nc.vector.memset(sc_pad[64:128, 1:2], 1.0)
        bqp = persist.tile([128, 2, 2], F32, tag="bqp")   # [p, m, parity]
        bkp = persist.tile([128, 2, 2], F32, tag="bkp")

        # ones column of V_aug, written once for all key tiles
        nc.vector.memset(
            v_aug[:, :, :, DK:DK + 1].bitcast(mybir.dt.uint32),
            ONES_F32_BITS)
        nc.sync.dma_start(ident[:], ident_in)
        nc.sync.dma_start(bv_bc[:], bv_bc_in)
        nc.sync.dma_start(bq_sb[:], b_q.rearrange("(m p) -> p m", p=128))
        nc.sync.dma_start(bk_sb[:], b_k.rearrange("(m p) -> p m", p=128))
        nc.sync.dma_start(mask_sb[:], mask_bias.rearrange("(t p) -> p t", p=128))
        for bp, bsrc in ((bqp, bq_sb), (bkp, bk_sb)):
            nc.vector.memset(bp[:], 0.0)
            for m in range(2):
                nc.vector.tensor_copy(bp[0:64, m, 0:1], bsrc[0:64, m:m + 1])
                nc.vector.tensor_copy(bp[64:128, m, 1:2],
                                      bsrc[64:128, m:m + 1])

        # ---------- phase A: x^T, QKV projections ----------
        with ExitStack() as pa:
            wqkv = pa.enter_context(tc.tile_pool(name="wqkv", bufs=1))
            xload = pa.enter_context(tc.tile_pool(name="xload", bufs=3))
            xtp = pa.enter_context(tc.tile_pool(name="xtp", bufs=2))
            tps = pa.enter_context(tc.tile_pool(name="tps", bufs=3, space="PSUM"))
            qps = pa.enter_context(tc.tile_pool(name="qps", bufs=3, space="PSUM"))
            vps = pa.enter_context(tc.tile_pool(name="vps", bufs=2, space="PSUM"))

            wq_sb = wqkv.tile([128, KT, DH], F32R, tag="wq")
            wk_sb = wqkv.tile([128, KT, DH], F32R, tag="wk")
            wv_sb = wqkv.tile([128, KT, DH], F32R, tag="wv")
            # weight loads: fast f32 HWDGE DMA into staging, DVE copy
            # rounds to f32r (f32r-typed DMAs hit a slow DGE path;
            # gpsimd cast-DMA is also slow)
            wo_stage = wqkv.tile([128, 2, D], F32, tag="wostage")
            nc.sync.dma_start(wo_stage[:],
                              w_out.rearrange("(t p) d -> p t d", p=128))
            nc.vector.tensor_copy(w_out_sb[:], wo_stage[:])
            for wi, (wt, wd) in enumerate(((wq_sb, w_q), (wk_sb, w_k),
                                           (wv_sb, w_v))):
                w_stage = wqkv.tile([128, KT, DH], F32, tag="wstage",
                                    name=f"wstage{wi}")
                nc.sync.dma_start(w_stage[:],
                                  wd.rearrange("(t p) d -> p t d", p=128))
                nc.vector.tensor_copy(wt[:], w_stage[:])

            for sc in range(SC if phases >= 0.6 else 0):
                # transpose 512 rows of x into xt_buf [128, kt, 512] (f32r)
                xt_buf = xtp.tile([128, KT, 512], F32R, tag="xt")
                for st4 in range(4):
                    sti = sc * 4 + st4
                    x_sb = xload.tile([128, D], F32, tag="x")
                    nc.sync.dma_start(x_sb[:], x[sti * 128:(sti + 1) * 128, :])
                    for kg in range(2):  # 2 groups of 4 transposes per psum bank
                        p_t = tps.tile([128, 4, 128], F32, tag="pt")
                        for kj in range(4):
                            k = kg * 4 + kj
                            nc.tensor.transpose(
                                p_t[:, kj, :],
                                x_sb[:, k * 128:(k + 1) * 128], ident[:])
                        cp = nc.any.tensor_copy if any_copy else nc.vector.tensor_copy
                        cp(out=xt_buf[:, kg * 4:(kg + 1) * 4,
                                      st4 * 128:(st4 + 1) * 128],
                           in_=p_t[:])

                # Q^T, K^T for this 512-wide s-chunk
                for (wt, bt, dst) in (((wq_sb, bqp, qt), (wk_sb, bkp, kt_sb))
                                      if phases >= 0.8 else ()):
                    for m in range(2):
                        p_q = qps.tile([128, 512], F32, tag="pq")
                        for k in range(KT):
                            nc.tensor.matmul(
                                p_q[:], wt[:, k, m * 128:(m + 1) * 128],
                                xt_buf[:, k, :],
                                start=(k == 0), stop=(k == KT - 1))
                        for a in range(2):
                            h = 2 * m + a
                            nc.scalar.activation(
                                dst[:, h, sc * 512:(sc + 1) * 512],
                                p_q[:], AF.Identity,
                                bias=bt[:, m, a:a + 1],
                                scale=sc_pad[:, a:a + 1])

                # V for the 4 s-tiles of this chunk
                for st4 in range(4 if phases >= 1 else 0):
                    sti = sc * 4 + st4
                    p_v = vps.tile([128, DH], F32, tag="pv")
                    for k in range(KT):
                        nc.tensor.matmul(
                            p_v[:], xt_buf[:, k, st4 * 128:(st4 + 1) * 128],
                            wv_sb[:, k, :],
                            start=(k == 0), stop=(k == KT - 1))
                    nc.vector.tensor_add(
                        v_aug[:, sti, :, 0:DK],
                        p_v[:].rearrange("p (h d) -> p h d", h=H_LOC),
                        bv_bc[:].rearrange("p (h d) -> p h d", h=H_LOC))

        # ---------- phase B: attention, head pairs interleaved ----------
        # Heads A (partitions 0:64) and B (64:128) of one K^T/Q^T tile are
        # issued alternately: their K=64 score matmuls target different PE
        # row groups (base_partition 0 / 64) and run concurrently, and the
        # per-head PSUM score buffers alternate so exp (ACT) overlaps the
        # next head's matmuls.
        with ExitStack() as pb:
            hp_range = range(2 if phases >= 2 else 0)
            epool = pb.enter_context(tc.tile_pool(name="expt", bufs=6))
            small = pb.enter_context(tc.tile_pool(name="small", bufs=6))
            ocopy = pb.enter_context(tc.tile_pool(name="ocopy", bufs=6))
            sps = pb.enter_context(tc.tile_pool(name="sps", bufs=1, space="PSUM"))
            ops = pb.enter_context(tc.tile_pool(name="ops", bufs=4, space="PSUM"))

            for hm in hp_range:          # K^T / Q^T partition tile = head pair
                for qh in range(2):      # q half [qh*1024, qh*1024+1024)
                    # po[a][j]: accumulator for head a, q chunk qh*1024+j*512
                    po = [[ops.tile([DK + 1, 512], F32, tag="po",
                                    name=f"po_{hm}_{qh}_{a}_{j}")
                           for j in range(2)] for a in range(2)]
                    for kti in range(ST):
                        e_ts = []
                        for a in range(2):   # head A then B, interleaved
                            h = 2 * hm + a
                            lhs_s = kt_sb[:, h, kti * 128:(kti + 1) * 128]
                            p_s = sps.tile([128, 1024], F32, tag=f"ps{a}",
                                           name=f"ps{a}")
                            for j in range(2):
                                q0 = qh * 1024 + j * 512
                                nc.tensor.matmul(
                                    p_s[:, j * 512:(j + 1) * 512], lhs_s,
                                    qt[:, h, q0:q0 + 512],
                                    start=True, stop=True)
                            e_t = epool.tile([128, 1024], F32R, tag="et",
                                             name=f"et{a}")
                            nc.scalar.activation(
                                e_t[:], p_s[:], AF.Exp,
                                bias=mask_sb[:, kti:kti + 1], scale=INV_SCALE)
                            e_ts.append(e_t)
                        for a in range(2):
                            h = 2 * hm + a
                            for j in range(2):
                                nc.tensor.matmul(
                                    po[a][j][:],
                                    v_aug[:, kti, h, :],
                                    e_ts[a][:, j * 512:(j + 1) * 512],
                                    start=(kti == 0), stop=(kti == ST - 1),
                                    skip_group_check=True)
                    # copy accumulators out to free PSUM banks, then normalize
                    for a in range(2):
                        hp = 64 * a
                        for j in range(2):
                            oc = ocopy.tile([DK + 1, 512], F32, tag="oc",
                                            name=f"oc{a}{j}")
                            nc.vector.tensor_copy(oc[:], po[a][j][:])
                            r_sb = small.tile([1, 512], F32, tag="rsb",
                                              name=f"rsb{a}{j}")
                            nc.vector.reciprocal(r_sb[0:1, :],
                                                 oc[DK:DK + 1, :])
                            bc_sb = small.tile([64, 512], F32, tag="bcsb",
                                               name=f"bcsb{a}{j}")
                            nc.gpsimd.partition_broadcast(
                                bc_sb[:], r_sb[0:1, :], channels=64)
                            q0 = qh * 1024 + j * 512
                            nc.vector.tensor_mul(
                                out_ht[hp:hp + 64, hm, q0:q0 + 512],
                                oc[0:DK, :], bc_sb[:])

        # ---------- phase C: output projection ----------
        with ExitStack() as pc:
            st_range = range(ST if phases >= 3 else 0)
            ypool = pc.enter_context(tc.tile_pool(name="ypool", bufs=4))
            yps = pc.enter_context(tc.tile_pool(name="yps", bufs=6, space="PSUM"))

            for sti in st_range:
                y_sb = ypool.tile([128, D], F32, tag="ysb")
                for m in range(2):
                    p_y = yps.tile([128, 512], F32, tag="py")
                    for k2 in range(2):
                        nc.tensor.matmul(
                            p_y[:], out_ht[:, k2, sti * 128:(sti + 1) * 128],
                            w_out_sb[:, k2, m * 512:(m + 1) * 512],
                            start=(k2 == 0), stop=(k2 == 1))
                    # b_out is added host-side during the unshard sum
                    cp = nc.any.tensor_copy if any_copy else nc.vector.tensor_copy
                    cp(out=y_sb[:, m * 512:(m + 1) * 512], in_=p_y[:])
                nc.sync.dma_start(y[sti * 128:(sti + 1) * 128, :], y_sb[:])

    nc.compile()
    return nc


def kernel(x, mask, W_qkv, b_qkv, W_out, b_out):
    global _CACHED_NC, LAST_EXEC_NS, LAST_RESULTS, LAST_IN_MAPS
    x = np.ascontiguousarray(np.asarray(x, dtype=np.float32))
    mask = np.asarray(mask)
    W_qkv = np.asarray(W_qkv, dtype=np.float32)
    b_qkv = np.asarray(b_qkv, dtype=np.float32)
    W_out = np.ascontiguousarray(np.asarray(W_out, dtype=np.float32))
    b_out_full = np.asarray(b_out, dtype=np.float32)

    B = x.shape[0]
    if _CACHED_NC is None:
        _CACHED_NC = _build()
    nc = _CACHED_NC

    mask_bias = ((mask.astype(np.float32) - 1.0) * 1e9).astype(np.float32)
    ident = np.eye(128, dtype=np.float32)
    bout_bc = np.broadcast_to(b_out_full, (128, D)).copy()
    bout_zero = np.zeros((128, D), dtype=np.float32)

    in_maps = []
    for c in range(8):
        b = c // 4
        g = c % 4
        cs = g * DH
        in_maps.append({
            "x": x[b],
            "ident": ident,
            "bv_bc": np.broadcast_to(
                b_qkv[2 * D + cs:2 * D + cs + DH], (128, DH)).copy(),
            "bout_bc": bout_bc if g == 0 else bout_zero,
            "w_q": np.ascontiguousarray(W_qkv[:, cs:cs + DH]),
            "w_k": np.ascontiguousarray(W_qkv[:, D + cs:D + cs + DH]),
            "w_v": np.ascontiguousarray(W_qkv[:, 2 * D + cs:2 * D + cs + DH]),
            "b_q": np.ascontiguousarray(b_qkv[cs:cs + DH]),
            "b_k": np.ascontiguousarray(b_qkv[D + cs:D + cs + DH]),
            "b_v": np.ascontiguousarray(b_qkv[2 * D + cs:2 * D + cs + DH]),
            "w_out": np.ascontiguousarray(W_out[cs:cs + DH, :]),
            "b_out": b_out_full if g == 0 else np.zeros_like(b_out_full),
            "mask_bias": mask_bias[b],
        })

    kwargs = {}
    if TRACE:
        kwargs["trace"] = True
        if TRACE_ALL_CORES:
            kwargs["trace_cores"] = list(range(8))
    LAST_IN_MAPS = in_maps
    res = None
    for attempt in range(3):
        try:
            res = run_bass_kernel_spmd(nc, in_maps, core_ids=list(range(8)),
                                       **kwargs)
            break
        except Exception:
            if attempt == 2:
                raise
            import time as _time
            _time.sleep(2.0)
    LAST_EXEC_NS = res.exec_time_ns
    LAST_RESULTS = res

    out = np.zeros((B, S, D), dtype=np.float32)
    for c in range(8):
        out[c // 4] += res.results[c]["y"]
    out += b_out_full
    return out

